# revision 7
# baseline (speedup 1.0000x reference)
"""Trainium2 Bass kernel for:
    tanh( (x0*x1 + sin(x2)) * exp(-|x3|) + x4 / (x5*x5 + exp(x6)) - x7 )
over inputs (8388608, 8) f32, data-parallel over 8 NeuronCores.

Design notes:
  - Rows sharded 8-way across cores (pure data parallel).
  - Per core: 1,048,576 rows -> 16 tiles of (128 partitions x 512 rows).
    Each tile's input is a contiguous 2MB DRAM block, DMA'd as
    (128, 4096) f32; per-variable views are stride-8 APs in the free dim.
  - ACT (ScalarE) table sets: `sin` only coexists with tanh/abs/square in
    the silu/trig sets; `exp` lives in exp_and_others (with tanh).
    Table switches cost ~2.7us, so tiles are processed in batches of B=4:
    all sins of a batch first (one set), then all exp/tanh work (other
    set) -> 2 switches per batch instead of 2 per tile.  Explicit
    same-engine ordering edges keep the scheduler from interleaving.
  - Division via the custom-DVE reciprocal_approx_fast (~51 ULP, 1 op).
  - abs(x3) (via abs_max(x,0)) and x5*x5 run on GPSIMD to off-load the
    two hottest engines (DVE/ACT).
"""

import numpy as np

import concourse.bass as bass
import concourse.bacc as bacc
import concourse.mybir as mybir
from concourse.tile import TileContext
from concourse.tile_rust import add_dep_helper
from concourse import bass_utils

N_ROWS = 8_388_608
N_VARS = 8
N_CORES = 8
ROWS_PER_CORE = N_ROWS // N_CORES  # 1_048_576
P = 128          # SBUF partitions
F = 512          # rows per partition per tile
TILE_ROWS = P * F                  # 65_536
N_TILES = ROWS_PER_CORE // TILE_ROWS  # 16
B = 4            # tiles per ACT-table batch

F32 = mybir.dt.float32
AF = mybir.ActivationFunctionType
OP = mybir.AluOpType


def build_bass(dep_edges: bool = True, use_gpsimd: bool = True,
               n_tiles: int = N_TILES, b: int = B) -> bass.Bass:
    nc = bacc.Bacc("TRN2", debug=False, num_devices=N_CORES)
    x = nc.dram_tensor("x", [ROWS_PER_CORE, N_VARS], F32, kind="ExternalInput").ap()
    y = nc.dram_tensor("y", [ROWS_PER_CORE], F32, kind="ExternalOutput").ap()

    with TileContext(nc) as tc:
        with (
            tc.tile_pool(name="inp", bufs=b + 2) as inp_pool,
            tc.tile_pool(name="sinp", bufs=b + 2) as sin_pool,
            tc.tile_pool(name="tmp", bufs=3) as tmp_pool,
        ):
            prev_batch_last_tanh = None
            for batch_start in range(0, n_tiles, b):
                batch = list(range(batch_start, min(batch_start + b, n_tiles)))

                # ---- Phase S: load inputs, sin(x2) (sin table set) ----
                staged = []
                sin_insts = []
                for t in batch:
                    r0, r1 = t * TILE_ROWS, (t + 1) * TILE_ROWS
                    xt = inp_pool.tile([P, F * N_VARS], F32, name=f"xt{t}", tag="xt")
                    nc.sync.dma_start(
                        out=xt,
                        in_=x[r0:r1, :].rearrange("(p f) v -> p (f v)", p=P),
                    )
                    xv = xt.rearrange("p (f v) -> p f v", v=N_VARS)
                    st = sin_pool.tile([P, F], F32, name=f"st{t}", tag="st")
                    si = nc.scalar.activation(st, xv[:, :, 2], AF.Sin)
                    if dep_edges and prev_batch_last_tanh is not None:
                        # keep ACT phases contiguous across batches
                        add_dep_helper(si.ins, prev_batch_last_tanh, False,
                                       "act-set phase order")
                    sin_insts.append(si.ins)
                    staged.append((t, xt, xv, st))

                last_sin = sin_insts[-1]

                # ---- Phase E: everything else (exp_and_others set) ----
                for t, xt, xv, st in staged:
                    r0, r1 = t * TILE_ROWS, (t + 1) * TILE_ROWS
                    a = tmp_pool.tile([P, F], F32, name=f"a{t}", tag="a")
                    bb = tmp_pool.tile([P, F], F32, name=f"bb{t}", tag="bb")
                    c = tmp_pool.tile([P, F], F32, name=f"c{t}", tag="c")
                    e = tmp_pool.tile([P, F], F32, name=f"e{t}", tag="e")
                    f = tmp_pool.tile([P, F], F32, name=f"f{t}", tag="f")
                    sq = tmp_pool.tile([P, F], F32, name=f"sq{t}", tag="sq")
                    e6 = tmp_pool.tile([P, F], F32, name=f"e6{t}", tag="e6")
                    d = tmp_pool.tile([P, F], F32, name=f"d{t}", tag="d")
                    rc = tmp_pool.tile([P, F], F32, name=f"rc{t}", tag="rc")
                    q = tmp_pool.tile([P, F], F32, name=f"q{t}", tag="q")
                    r = tmp_pool.tile([P, F], F32, name=f"r{t}", tag="r")
                    u = tmp_pool.tile([P, F], F32, name=f"u{t}", tag="u")
                    o = tmp_pool.tile([P, F], F32, name=f"o{t}", tag="o")

                    # x5*x5 (GPSIMD offloads DVE when enabled)
                    eng = nc.gpsimd if use_gpsimd else nc.vector
                    eng.tensor_tensor(
                        out=sq, in0=xv[:, :, 5], in1=xv[:, :, 5], op=OP.mult
                    )

                    # ACT: c=|x3| (Abs is in every table set), e = exp(-c),
                    # e6 = exp(x6)  (exp_and_others)
                    nc.scalar.activation(c, xv[:, :, 3], AF.Abs)
                    i1 = nc.scalar.activation(e, c, AF.Exp, scale=-1.0)
                    i2 = nc.scalar.activation(e6, xv[:, :, 6], AF.Exp)
                    if dep_edges:
                        for bi in (i1, i2):
                            add_dep_helper(bi.ins, last_sin, False,
                                           "act-set phase order")

                    # DVE chain
                    nc.vector.tensor_tensor(out=a, in0=xv[:, :, 0], in1=xv[:, :, 1],
                                            op=OP.mult)          # x0*x1
                    nc.vector.tensor_add(out=bb, in0=a, in1=st)  # + sin(x2)
                    nc.vector.tensor_tensor(out=f, in0=bb, in1=e, op=OP.mult)
                    nc.vector.tensor_add(out=d, in0=sq, in1=e6)  # x5^2+e^x6
                    nc.vector.reciprocal_approx_fast(out=rc, in_=d)
                    nc.vector.tensor_tensor(out=q, in0=xv[:, :, 4], in1=rc,
                                            op=OP.mult)          # q = x4/d
                    nc.vector.tensor_add(out=r, in0=f, in1=q)
                    nc.vector.tensor_tensor(out=u, in0=r, in1=xv[:, :, 7],
                                            op=OP.subtract)

                    i3 = nc.scalar.activation(o, u, AF.Tanh)
                    if dep_edges:
                        add_dep_helper(i3.ins, last_sin, False,
                                       "act-set phase order")
                    prev_batch_last_tanh = i3.ins

                    nc.sync.dma_start(
                        out=y[r0:r1].rearrange("(p f) -> p f", p=P),
                        in_=o,
                    )
    nc.compile()
    return nc


_BUILT = None


def _get_built():
    global _BUILT
    if _BUILT is None:
        _BUILT = build_bass()
    return _BUILT


def run_spmd(inputs: np.ndarray, **kwargs) -> tuple[np.ndarray, object]:
    """Shard, run on 8 cores, gather.  Returns (full output, BassKernelResults)."""
    x = np.ascontiguousarray(np.asarray(inputs, dtype=np.float32))
    assert x.shape == (N_ROWS, N_VARS), x.shape
    shards = x.reshape(N_CORES, ROWS_PER_CORE, N_VARS)
    in_maps = [{"x": np.ascontiguousarray(shards[i])} for i in range(N_CORES)]
    nc = _get_built()
    res = bass_utils.run_bass_kernel_spmd(
        nc, in_maps, core_ids=list(range(N_CORES)), **kwargs
    )
    out = np.concatenate([r["y"].reshape(-1) for r in res.results], axis=0)
    return out, res


def kernel(inputs: np.ndarray) -> np.ndarray:
    out, _ = run_spmd(inputs)
    return out


# revision 23
# speedup vs baseline: 41199.2265x; 41199.2265x over previous
"""Trainium2 Bass kernel for:
    tanh( (x0*x1 + sin(x2)) * exp(-|x3|) + x4 / (x5*x5 + exp(x6)) - x7 )
over inputs (8388608, 8) f32, data-parallel over 8 NeuronCores.

Design notes:
  - Rows sharded 8-way across cores (pure data parallel).
  - Per core: 1,048,576 rows -> 16 tiles of (128 partitions x 512 rows).
    Each tile's input is a contiguous 2MB DRAM block, DMA'd as
    (128, 4096) f32; per-variable views are stride-8 APs in the free dim.
  - ACT (ScalarE) table sets: `sin` only coexists with tanh/abs/square in
    the silu/trig sets; `exp` lives in exp_and_others (with tanh).
    Table switches cost ~2.7us, so tiles are processed in batches of B=4:
    all sins of a batch first (one set), then all exp/tanh work (other
    set) -> 2 switches per batch instead of 2 per tile.  Explicit
    same-engine ordering edges keep the scheduler from interleaving.
  - Division via the custom-DVE reciprocal_approx_fast (~51 ULP, 1 op).
  - abs(x3) (via abs_max(x,0)) and x5*x5 run on GPSIMD to off-load the
    two hottest engines (DVE/ACT).
"""

import numpy as np

import concourse.bass as bass
import concourse.bacc as bacc
import concourse.mybir as mybir
from concourse.tile import TileContext
from concourse.tile_rust import add_dep_helper
from concourse import bass_utils

N_ROWS = 8_388_608
N_VARS = 8
N_CORES = 8
ROWS_PER_CORE = N_ROWS // N_CORES  # 1_048_576
P = 128          # SBUF partitions
F = 512          # rows per partition per tile
TILE_ROWS = P * F                  # 65_536
N_TILES = ROWS_PER_CORE // TILE_ROWS  # 16
B = 4            # tiles per ACT-table batch

F32 = mybir.dt.float32
AF = mybir.ActivationFunctionType
OP = mybir.AluOpType


def build_bass(dep_edges: bool = True, use_gpsimd: bool = True,
               n_tiles: int = N_TILES, b: int = B,
               k_iters: int = 1, loop_iters: int = 1,
               ablate: str = "none") -> bass.Bass:
    """ablate: 'none' | 'dma' (no compute) | 'nodve' | 'noact' —
    wrong results, used only to attribute time between engines."""
    import contextlib
    nc = bacc.Bacc("TRN2", debug=False, num_devices=N_CORES)
    x = nc.dram_tensor("x", [ROWS_PER_CORE, N_VARS], F32, kind="ExternalInput").ap()
    y = nc.dram_tensor("y", [ROWS_PER_CORE], F32, kind="ExternalOutput").ap()

    with TileContext(nc) as tc:
        with (
            tc.tile_pool(name="inp", bufs=b + 2) as inp_pool,
            tc.tile_pool(name="sinp", bufs=b + 2) as sin_pool,
            tc.tile_pool(name="tmp", bufs=3) as tmp_pool,
            (tc.For_i(0, loop_iters, 1) if loop_iters > 1
             else contextlib.nullcontext()),
        ):
            prev_batch_last_tanh = None
            for batch_start in [s for _ in range(k_iters)
                                for s in range(0, n_tiles, b)]:
                batch = list(range(batch_start, min(batch_start + b, n_tiles)))

                # ---- Phase S: load inputs, sin(x2) (sin table set) ----
                staged = []
                sin_insts = []
                for t in batch:
                    r0, r1 = t * TILE_ROWS, (t + 1) * TILE_ROWS
                    xt = inp_pool.tile([P, F * N_VARS], F32, name=f"xt{t}", tag="xt")
                    nc.sync.dma_start(
                        out=xt,
                        in_=x[r0:r1, :].rearrange("(p f) v -> p (f v)", p=P),
                    )
                    xv = xt.rearrange("p (f v) -> p f v", v=N_VARS)
                    if ablate == "dma":
                        nc.sync.dma_start(
                            out=y[r0:r1].rearrange("(p f) -> p f", p=P),
                            in_=xt[:, 0:F],
                        )
                        continue
                    st = sin_pool.tile([P, F], F32, name=f"st{t}", tag="st")
                    # ACT's sin spline is only accurate on [-pi, pi]; inputs
                    # reach |x2|~5.5, so wrap by one period first (DVE).
                    wr = sin_pool.tile([P, F], F32, name=f"wr{t}", tag="wr")
                    if ablate != "nodve":
                        nc.vector.add_range_wrap(
                            out=wr, in_=xv[:, :, 2], shift=0.0,
                            bound=float(np.pi), period=float(2 * np.pi),
                        )
                    si = None
                    if ablate != "noact":
                        src = xv[:, :, 2] if ablate == "nodve" else wr
                        si = nc.scalar.activation(st, src, AF.Sin)
                        if dep_edges and prev_batch_last_tanh is not None:
                            # keep ACT phases contiguous across batches
                            add_dep_helper(si.ins, prev_batch_last_tanh, False,
                                           "act-set phase order")
                        sin_insts.append(si.ins)
                    staged.append((t, xt, xv, st, wr))

                last_sin = sin_insts[-1] if sin_insts else None
                if ablate == "dma":
                    continue

                # ---- Phase E: everything else (exp_and_others set) ----
                for t, xt, xv, st, wr in staged:
                    r0, r1 = t * TILE_ROWS, (t + 1) * TILE_ROWS
                    a = tmp_pool.tile([P, F], F32, name=f"a{t}", tag="a")
                    bb = tmp_pool.tile([P, F], F32, name=f"bb{t}", tag="bb")
                    cc = tmp_pool.tile([P, F], F32, name=f"cc{t}", tag="cc")
                    e = tmp_pool.tile([P, F], F32, name=f"e{t}", tag="e")
                    f = tmp_pool.tile([P, F], F32, name=f"f{t}", tag="f")
                    sq = tmp_pool.tile([P, F], F32, name=f"sq{t}", tag="sq")
                    e6 = tmp_pool.tile([P, F], F32, name=f"e6{t}", tag="e6")
                    d = tmp_pool.tile([P, F], F32, name=f"d{t}", tag="d")
                    rc = tmp_pool.tile([P, F], F32, name=f"rc{t}", tag="rc")
                    q = tmp_pool.tile([P, F], F32, name=f"q{t}", tag="q")
                    r = tmp_pool.tile([P, F], F32, name=f"r{t}", tag="r")
                    u = tmp_pool.tile([P, F], F32, name=f"u{t}", tag="u")
                    o = tmp_pool.tile([P, F], F32, name=f"o{t}", tag="o")

                    # GPSIMD: x5*x5 — the same-AP strided mult is cheap on
                    # Pool (~0.2us measured); copies there are NOT (~5us).
                    nc.gpsimd.tensor_tensor(
                        out=sq, in0=xv[:, :, 5], in1=xv[:, :, 5], op=OP.mult)

                    # ACT: cc=|x3| (Abs is in every table set), e=exp(-cc),
                    # e6=exp(x6)   (exp_and_others)
                    nc.scalar.activation(cc, xv[:, :, 3], AF.Abs)
                    i1 = nc.scalar.activation(e, cc, AF.Exp, scale=-1.0)
                    i2 = nc.scalar.activation(e6, xv[:, :, 6], AF.Exp)
                    if dep_edges and last_sin is not None:
                        for bi in (i1, i2):
                            add_dep_helper(bi.ins, last_sin, False,
                                           "act-set phase order")

                    # DVE chain
                    nc.vector.tensor_tensor(out=a, in0=xv[:, :, 0],
                                            in1=xv[:, :, 1],
                                            op=OP.mult)          # x0*x1
                    nc.vector.tensor_add(out=bb, in0=a, in1=st)  # + sin(x2)
                    nc.vector.tensor_tensor(out=f, in0=bb, in1=e, op=OP.mult)
                    nc.vector.tensor_add(out=d, in0=sq, in1=e6)  # x5^2+e^x6
                    nc.vector.reciprocal_approx_fast(out=rc, in_=d)
                    nc.vector.tensor_tensor(out=q, in0=xv[:, :, 4], in1=rc,
                                            op=OP.mult)          # q = x4/d
                    nc.vector.tensor_add(out=r, in0=f, in1=q)
                    nc.vector.tensor_tensor(out=u, in0=r, in1=xv[:, :, 7],
                                            op=OP.subtract)

                    i3 = nc.scalar.activation(o, u, AF.Tanh)
                    if dep_edges and last_sin is not None:
                        add_dep_helper(i3.ins, last_sin, False,
                                       "act-set phase order")
                    prev_batch_last_tanh = i3.ins

                    nc.sync.dma_start(
                        out=y[r0:r1].rearrange("(p f) -> p f", p=P),
                        in_=o,
                    )
    nc.compile()
    return nc


_BUILT = None


def _get_built():
    global _BUILT
    if _BUILT is None:
        _BUILT = build_bass()
    return _BUILT


def run_spmd(inputs: np.ndarray, **kwargs) -> tuple[np.ndarray, object]:
    """Shard, run on 8 cores, gather.  Returns (full output, BassKernelResults)."""
    x = np.ascontiguousarray(np.asarray(inputs, dtype=np.float32))
    assert x.shape == (N_ROWS, N_VARS), x.shape
    shards = x.reshape(N_CORES, ROWS_PER_CORE, N_VARS)
    in_maps = [{"x": np.ascontiguousarray(shards[i])} for i in range(N_CORES)]
    nc = _get_built()
    res = bass_utils.run_bass_kernel_spmd(
        nc, in_maps, core_ids=list(range(N_CORES)), **kwargs
    )
    out = np.concatenate([r["y"].reshape(-1) for r in res.results], axis=0)
    return out, res


def kernel(inputs: np.ndarray) -> np.ndarray:
    out, _ = run_spmd(inputs)
    return out
